# revision 14
# baseline (speedup 1.0000x reference)
"""Trainium2 Bass kernel for a GQA attention layer (B=2, S=2048, D=4096,
32 q-heads, 8 kv-heads, HD=128, RoPE, causal mask).

Sharding: 8 cores = 2 (batch) x 4 (head groups). Each core handles one
batch and 8 q-heads / 2 kv-heads: column-parallel wq/wk/wv, row-parallel
wo. Each core emits a partial [S, D] output (bf16); the host sums the 4
partials per batch in f32. No collectives.

Device dataflow (per core), v2:
  phase 1: QKV projections from host-pretransposed xT (feature-major),
           RoPE applied in a "split" head layout. K chains first so
           attention on the next panel unblocks early.
  phase 2: scoresT[sk,sq] = K^T-tiles (stationary) x Q^T (moving); the
           causal-diagonal mask is added ON THE PE via a chained
           identity-stationary matmul (no DVE in the PE->ACT path); exp
           on ScalarE with scale=1/sqrt(HD). Softmax denominator: DVE
           accumulates the prob tiles (bf16 2x rate) into acc[128,512],
           then ONE ones-stationary matmul per (head, block) broadcasts
           the cross-partition sum - removing the per-tile denominator
           matmul stream from the PE entirely. attnT accumulated with V
           (token-major) stationary; 1/denom applied during psum
           evacuation. Finalize (ones-MM + reciprocal + multiply) is
           deferred into the next head's stream so the PE never waits
           on the DVE accumulation.
  phase 3: out_partial = attnT^T x wo-rows. m-tiles 0..11 (which only
           need attention blocks 0..2) are emitted interleaved with
           attention block 3; m-tiles 12..15 follow with reloaded wo.

  PSUM pools are split per stream (proj 2 / scores 3 / attnV+denom 3
  banks) so the tile scheduler can overlap the projection, attention and
  output-projection instruction streams without ring-buffer coupling.
"""

import sys

if "/opt/trn_rl_repo" not in sys.path:
    sys.path.insert(0, "/opt/trn_rl_repo")

import math
from contextlib import ExitStack

import ml_dtypes
import numpy as np

import concourse.bass as bass  # noqa: F401  (AP types used implicitly)
import concourse.tile as tile
from concourse import bacc, mybir
from concourse.bass_utils import run_bass_kernel_spmd

BF16 = ml_dtypes.bfloat16
F32 = mybir.dt.float32
BF = mybir.dt.bfloat16

B, S, D = 2, 2048, 4096
NH, NKV, HD = 32, 8, 128
G = 4  # head groups -> cores per batch
HPG = NH // G  # 8 q heads per core
KPG = NKV // G  # 2 kv heads per core
SCALE = 1.0 / math.sqrt(HD)

NFT = D // 128  # 32 feature tiles (contraction)
PTOK = 512  # token panel width in phase 1
NPANEL = S // PTOK  # 4
NTT = S // 128  # 16 token tiles
NSQ = S // 512  # 4 sq tiles
NOD = D // 512  # 8 out-D tiles

_CACHE = {}


def _build_program(phases=(1, 2, 3), reps=1):
    nc = bacc.Bacc("TRN2", target_bir_lowering=False, debug=False, num_devices=8)

    # x pre-packed on host: [panel*4+quarter, 128, 8*512] so every panel DMA
    # is fully contiguous (8KB per partition line)
    xt = nc.dram_tensor("xt", [4 * NPANEL, 128, 8 * PTOK], BF, kind="ExternalInput").ap()
    wq = nc.dram_tensor("wq", [HPG, 128, NFT * 128], BF, kind="ExternalInput").ap()
    wk = nc.dram_tensor("wk", [KPG, 128, NFT * 128], BF, kind="ExternalInput").ap()
    wv = nc.dram_tensor("wv", [128, NFT * KPG * 128], BF, kind="ExternalInput").ap()
    wo = nc.dram_tensor("wo", [NOD, 128, HPG * 512], BF, kind="ExternalInput").ap()
    cosb = nc.dram_tensor("cosb", [128, S], BF, kind="ExternalInput").ap()
    sinb = nc.dram_tensor("sinb", [128, S], BF, kind="ExternalInput").ap()
    diagm = nc.dram_tensor("diagm", [128, 128], BF, kind="ExternalInput").ap()
    ones = nc.dram_tensor("ones", [128, 128], BF, kind="ExternalInput").ap()
    ident = nc.dram_tensor("ident", [128, 128], BF, kind="ExternalInput").ap()
    outp = nc.dram_tensor("outp", [S, D], BF, kind="ExternalOutput").ap()

    EXP = mybir.ActivationFunctionType.Exp
    MULT = mybir.AluOpType.mult

    with tile.TileContext(nc) as tc, ExitStack() as ctx:
        pool = lambda name, bufs: ctx.enter_context(tc.tile_pool(name=name, bufs=bufs))
        ppool = lambda name, bufs: ctx.enter_context(
            tc.tile_pool(name=name, bufs=bufs, space="PSUM")
        )

        persist = pool("persist", 1)
        xpool = pool("xpool", 5)
        wqpool = pool("wqpool", 2)
        ropepool = pool("ropepool", 2)
        probpool = pool("probpool", 12)
        accpool = pool("accpool", 6)
        invpool = pool("invpool", 2)
        wopool = pool("wopool", 2)
        outpool = pool("outpool", 3)

        psProj = ppool("psProj", 2)  # [128,512] f32: QKV projections, O-proj
        psS = ppool("psS", 3)  # [128,512] f32: score tiles
        psA1 = ppool("psA1", 3)  # [128,512] f32: attnV accum + denominators

        # ---- persistent tiles ----
        qt = [persist.tile([128, S], BF, tag=f"qt{h}", name=f"qt{h}") for h in range(HPG)]
        kt = [persist.tile([128, S], BF, tag=f"kt{k}", name=f"kt{k}") for k in range(KPG)]
        v_sb = persist.tile([128, NTT * KPG * 128], BF, tag="v", name="v_sb")
        v_w_sb = persist.tile([128, NFT * KPG * 128], BF, tag="vw", name="v_w_sb")
        at = [persist.tile([128, S], BF, tag=f"at{h}", name=f"at{h}") for h in range(HPG)]
        cos_sb = persist.tile([128, S], BF, tag="cos", name="cos_sb")
        sin_sb = persist.tile([128, S], BF, tag="sin", name="sin_sb")
        diag_sb = persist.tile([128, 128], BF, tag="diag", name="diag_sb")
        ones_sb = persist.tile([128, 128], BF, tag="ones", name="ones_sb")
        ident_sb = persist.tile([128, 128], BF, tag="ident", name="ident_sb")

        nc.scalar.dma_start(cos_sb[:], cosb[:])
        nc.scalar.dma_start(sin_sb[:], sinb[:])
        nc.scalar.dma_start(diag_sb[:], diagm[:])
        nc.scalar.dma_start(ones_sb[:], ones[:])
        nc.scalar.dma_start(ident_sb[:], ident[:])
        nc.scalar.dma_start(v_w_sb[:], wv[:])

        do1, do2, do3 = (1 in phases), (2 in phases), (3 in phases)
        # K chains first so next-panel attention unblocks early.
        qk_dst = list(kt) + list(qt)
        qk_src = [wk[i] for i in range(KPG)] + [wq[i] for i in range(HPG)]

        # finalize closures deferred into the following instruction stream
        pending = []

        def flush_pending():
            while pending:
                pending.pop(0)()

        def load_x(n):
            halves = []
            for q4 in range(4):
                xq = xpool.tile([128, 8 * PTOK], BF, tag="xts", name="xq")
                nc.sync.dma_start(xq[:], xt[n * 4 + q4])
                halves.append((xq, q4 * 8))
            return halves

        def proj_chain(n, hh, halves):
            tok0 = n * PTOK
            wh = wqpool.tile([128, NFT * 128], BF, tag="wqt", name="wh")
            nc.scalar.dma_start(wh[:], qk_src[hh])
            ps = psProj.tile([128, PTOK], F32, tag="psP", name="ps_qk")
            for xtile, f0 in halves:
                for fl in range(8):
                    f = f0 + fl
                    nc.tensor.matmul(
                        ps[:],
                        wh[:, f * 128 : (f + 1) * 128],
                        xtile[:, fl * PTOK : (fl + 1) * PTOK],
                        start=(f == 0),
                        stop=(f == NFT - 1),
                    )
            dst = qk_dst[hh]
            nc.vector.tensor_copy(dst[:, tok0 : tok0 + PTOK], ps[:])
            # RoPE immediately after the evac: dst = dst*C + swap_halves(dst)*S2
            rsw = ropepool.tile([128, PTOK], BF, tag="rsw", name="rsw")
            nc.sync.dma_start(rsw[0:64, :], dst[64:128, tok0 : tok0 + PTOK])
            nc.sync.dma_start(rsw[64:128, :], dst[0:64, tok0 : tok0 + PTOK])
            nc.vector.tensor_mul(rsw[:], rsw[:], sin_sb[:, tok0 : tok0 + PTOK])
            nc.vector.tensor_mul(
                dst[:, tok0 : tok0 + PTOK],
                dst[:, tok0 : tok0 + PTOK],
                cos_sb[:, tok0 : tok0 + PTOK],
            )
            nc.vector.tensor_add(
                dst[:, tok0 : tok0 + PTOK],
                dst[:, tok0 : tok0 + PTOK],
                rsw[:],
            )

        def proj_v(n, m, halves):
            ps = psProj.tile([128, PTOK], F32, tag="psP", name="ps_v")
            for xtile, f0 in halves:
                for fl in range(8):
                    f = f0 + fl
                    nc.tensor.matmul(
                        ps[:, 0 : KPG * 128],
                        xtile[:, fl * PTOK + m * 128 : fl * PTOK + m * 128 + 128],
                        v_w_sb[:, f * 256 : (f + 1) * 256],
                        start=(f == 0),
                        stop=(f == NFT - 1),
                    )
            tglob = n * (PTOK // 128) + m
            nc.vector.tensor_copy(
                v_sb[:, tglob * 256 : (tglob + 1) * 256], ps[:, 0 : KPG * 128]
            )

        def attn_head(j, h):
            sq0 = j * 512
            n_sk = 4 * (j + 1)
            SKEW = 4
            kv = h // (HPG // KPG)
            # denominator accumulation is parity-split across gpsimd (even
            # tiles) and DVE (odd tiles) so neither strict-FIFO engine
            # serializes against the exp stream; the two partial accumulators
            # are folded by chained ones-matmuls in fin().
            acc_e = accpool.tile([128, 512], BF, tag="acc", name="acc_e")
            acc_o = accpool.tile([128, 512], BF, tag="acc", name="acc_o")
            # ps_d allocated up-front (not inside fin) so the psA1 ring never
            # waits on its own head's evacuation.
            ps_a = psA1.tile([128, 512], F32, tag="psA1", name="psAt_t")
            ps_d = psA1.tile([128, 512], F32, tag="psA1", name="psD_t")
            if j == 0:
                # block 0 has no full-width odd tile to initialize acc_o from
                nc.vector.memset(acc_o[:], 0.0)
            pts = {}
            flushed = False
            for tt in range(n_sk + SKEW):
                if tt == 2 and not flushed:
                    flushed = True
                    flush_pending()
                if tt < n_sk:
                    t = tt
                    r = t - 4 * j
                    off = 128 * r if r >= 0 else 0
                    ps_s = psS.tile([128, 512], F32, tag="psS", name="psS_t")
                    nc.tensor.matmul(
                        ps_s[:, off:512],
                        kt[kv][:, t * 128 : (t + 1) * 128],
                        qt[h][:, sq0 + off : sq0 + 512],
                        start=True,
                        stop=(r < 0),
                    )
                    if r >= 0:
                        # causal-diagonal mask added on the PE: psum += I^T @ diag
                        nc.tensor.matmul(
                            ps_s[:, off : off + 128],
                            ident_sb[:],
                            diag_sb[:],
                            start=False,
                            stop=True,
                        )
                    pt = probpool.tile([128, 512], BF, tag="probs", name="probs_t")
                    nc.scalar.activation(
                        pt[:, off:512], ps_s[:, off:512], EXP, scale=SCALE
                    )
                    pts[t] = (pt, off)
                if tt >= SKEW:
                    t = tt - SKEW
                    pt, off = pts.pop(t)
                    nc.tensor.matmul(
                        ps_a[:, off:512],
                        v_sb[:, t * 256 + kv * 128 : t * 256 + kv * 128 + 128],
                        pt[:, off:512],
                        start=(t == 0),
                        stop=(t == n_sk - 1),
                    )
                    eng = nc.gpsimd if t % 2 == 0 else nc.vector
                    acc = acc_e if t % 2 == 0 else acc_o
                    if t < 2 and off == 0:
                        eng.tensor_copy(acc[:], pt[:])
                    else:
                        eng.tensor_add(acc[:, off:512], acc[:, off:512], pt[:, off:512])

            def fin(h=h, acc_e=acc_e, acc_o=acc_o, ps_a=ps_a, ps_d=ps_d, sq0=sq0):
                nc.tensor.matmul(ps_d[:], ones_sb[:], acc_e[:], start=True, stop=False)
                nc.tensor.matmul(ps_d[:], ones_sb[:], acc_o[:], start=False, stop=True)
                inv = invpool.tile([128, 512], F32, tag="inv", name="inv")
                nc.vector.reciprocal(inv[:], ps_d[:])
                nc.vector.tensor_tensor(
                    at[h][:, sq0 : sq0 + 512], ps_a[:], inv[:], MULT
                )

            pending.append(fin)

        def oproj_unit(d, m, wod):
            ps = psProj.tile([128, 512], F32, tag="psP", name="ps_o")
            for h in range(HPG):
                nc.tensor.matmul(
                    ps[:],
                    at[h][:, m * 128 : (m + 1) * 128],
                    wod[:, h * 512 : (h + 1) * 512],
                    start=(h == 0),
                    stop=(h == HPG - 1),
                )
            osb = outpool.tile([128, 512], BF, tag="osb", name="osb")
            nc.vector.tensor_copy(osb[:], ps[:])
            nc.sync.dma_start(
                outp[m * 128 : (m + 1) * 128, d * 512 : (d + 1) * 512], osb[:]
            )

        def panel(n):
            """Panel n projections interleaved with attention block n-1."""
            halves = load_x(n)
            attn = do2 and n >= 1
            # the first two attention heads go ahead of the chains so the PE
            # has work while the panel's x tiles stream in
            if attn:
                attn_head(n - 1, 0)
                attn_head(n - 1, 1)
            for hh in range(HPG + KPG):
                proj_chain(n, hh, halves)
                if attn and 2 <= hh < 8:
                    attn_head(n - 1, hh)
            for m in range(PTOK // 128):
                proj_v(n, m, halves)

        for _rep in range(reps):
            if do1:
                panel(0)
                for n in range(1, NPANEL):
                    panel(n)
            elif do2:
                for n in range(1, NPANEL):
                    for h in range(HPG):
                        attn_head(n - 1, h)
            # tail: attention block 3 interleaved with o-proj m-tiles 0..11
            if do2 and not do3:
                for h in range(HPG):
                    attn_head(NPANEL - 1, h)
                flush_pending()
            if do3:
                ai = 0  # next attn3 head to emit
                for d in range(NOD):
                    wod = wopool.tile([128, HPG * 512], BF, tag="wot", name="wod")
                    nc.scalar.dma_start(wod[:], wo[d])
                    for m in range(12):
                        if do2 and ai < HPG and (d * 12 + m) % 11 == 1:
                            attn_head(NPANEL - 1, ai)
                            ai += 1
                        oproj_unit(d, m, wod)
                if do2:
                    while ai < HPG:
                        attn_head(NPANEL - 1, ai)
                        ai += 1
                    flush_pending()
                for d in range(NOD):
                    wod = wopool.tile([128, HPG * 512], BF, tag="wot", name="wod")
                    nc.scalar.dma_start(wod[:], wo[d])
                    for m in range(12, NTT):
                        oproj_unit(d, m, wod)
            flush_pending()

    nc.compile()
    return nc


_SPLIT_PERM = np.concatenate([np.arange(0, HD, 2), np.arange(1, HD, 2)])


def _host_prep(x, freqs_cos, freqs_sin, mask, wq, wk, wv, wo):
    """Build per-core input maps (8 cores = 2 batches x 4 head groups)."""
    x = np.asarray(x, np.float32)
    wq = np.asarray(wq, np.float32)
    wk = np.asarray(wk, np.float32)
    wv = np.asarray(wv, np.float32)
    wo = np.asarray(wo, np.float32)
    freqs_cos = np.asarray(freqs_cos, np.float32)
    freqs_sin = np.asarray(freqs_sin, np.float32)
    mask = np.asarray(mask, np.float32)

    # pack xT into [panel*4+quarter, 128, 8*512]: element (n*4+q4, p, f*512+t)
    # = x[b].T[(q4*8+f)*128 + p, n*512+t]
    xts = []
    for b in range(B):
        xtb = np.ascontiguousarray(x[b].T).astype(BF16)  # [D, S]
        x5 = xtb.reshape(4, 8, 128, NPANEL, PTOK)  # [q4, f, p, n, t]
        xts.append(
            np.ascontiguousarray(x5.transpose(3, 0, 2, 1, 4)).reshape(
                4 * NPANEL, 128, 8 * PTOK
            )
        )

    ct = freqs_cos.T  # [64, S]
    st = freqs_sin.T
    cosb = np.concatenate([ct, ct], axis=0).astype(BF16)
    sinb = np.concatenate([-st, st], axis=0).astype(BF16)
    diagm = np.ascontiguousarray(
        mask[0:128, 0:128].T * math.sqrt(HD), dtype=np.float32
    ).astype(BF16)
    ones = np.ones((128, 128), BF16)
    ident = np.eye(128, dtype=np.float32).astype(BF16)

    per_g = []
    for g in range(G):
        wq_g = wq[:, g * HPG * HD : (g + 1) * HPG * HD].reshape(D, HPG, HD)
        wq_g = wq_g[:, :, _SPLIT_PERM]
        wq_g = np.ascontiguousarray(
            wq_g.reshape(NFT, 128, HPG, HD).transpose(2, 1, 0, 3).reshape(HPG, 128, NFT * 128)
        ).astype(BF16)

        wk_g = wk[:, g * KPG * HD : (g + 1) * KPG * HD].reshape(D, KPG, HD)
        wk_g = wk_g[:, :, _SPLIT_PERM]
        wk_g = np.ascontiguousarray(
            wk_g.reshape(NFT, 128, KPG, HD).transpose(2, 1, 0, 3).reshape(KPG, 128, NFT * 128)
        ).astype(BF16)

        wv_g = np.ascontiguousarray(
            wv[:, g * KPG * HD : (g + 1) * KPG * HD]
            .reshape(NFT, 128, KPG * 128)
            .transpose(1, 0, 2)
            .reshape(128, NFT * KPG * 128)
        ).astype(BF16)

        wo_g = wo[g * HPG * HD : (g + 1) * HPG * HD, :]
        wo_g = np.ascontiguousarray(
            wo_g.reshape(HPG, 128, NOD, 512).transpose(2, 1, 0, 3).reshape(NOD, 128, HPG * 512)
        ).astype(BF16)

        per_g.append((wq_g, wk_g, wv_g, wo_g))

    in_maps = []
    for core in range(8):
        b, g = divmod(core, G)
        wq_g, wk_g, wv_g, wo_g = per_g[g]
        in_maps.append(
            {
                "xt": xts[b],
                "wq": wq_g,
                "wk": wk_g,
                "wv": wv_g,
                "wo": wo_g,
                "cosb": cosb,
                "sinb": sinb,
                "diagm": diagm,
                "ones": ones,
                "ident": ident,
            }
        )
    return in_maps


def get_program(phases=(1, 2, 3), reps=1):
    key = ("nc", tuple(phases), reps)
    if key not in _CACHE:
        _CACHE[key] = _build_program(phases, reps)
    return _CACHE[key]


def kernel(
    x, start_pos, freqs_cos, freqs_sin, mask, wq, wk, wv, wo, **_ignored
):
    nc = get_program()
    in_maps = _host_prep(x, freqs_cos, freqs_sin, mask, wq, wk, wv, wo)
    res = run_bass_kernel_spmd(nc, in_maps, core_ids=list(range(8)))
    partials = [res.results[c]["outp"].astype(np.float32) for c in range(8)]
    out = np.stack(
        [
            partials[b * G]
            + partials[b * G + 1]
            + partials[b * G + 2]
            + partials[b * G + 3]
            for b in range(B)
        ]
    ).astype(np.float32)
    return out
